# revision 35
# baseline (speedup 1.0000x reference)
"""ArcFace loss (PthArcLoss) Trainium2 Bass kernel (raw bass, no Tile).

Model-parallel over the class dimension: the [C, d] class-weight matrix is
sharded across 8 NeuronCores.  Each core computes its local logits with
fp8(e4m3) DoubleRow matmuls (s=64 folded as 8x into each normalized operand,
so PSUM accumulates s*cos directly in fp32) over 7 uniform 1792-class slots
x 4 batch-row tiles.  The softmax denominator is split per slot between two
engines running concurrently under the PE issue rate:

  - ScalarE exponentiates the first 1152 columns (exp LUT, fixed -64 shift)
    with accum_out producing the row-sum directly (one ~285ns
    ACTIVATION_READ_ACCUMULATOR per op).
  - VectorE handles the last 640 columns with a Schraudolph integer exp:
    one tensor_scalar (bits = z*A + C, f32->int32 convert on write) and one
    reduce_sum over the int32 tile bitcast to f32.  The piecewise-linear
    mantissa error (+-3%, bias-tuned) averages out across the ~100k summed
    terms; measured end loss error ~7e-5 relative.

Raw bass with hand-rolled semaphores (the Tile scheduler added ~9.5us of
post-compute teardown ladder).  Hard-won HW constraints baked in:
  - PE writes and ACT/DVE reads must never touch the same 4-bank PSUM half
    concurrently (mid-iteration reads, >2-deep rotations, and sub-512-slot
    variants all hang real hardware), so the pipeline is exactly 2-deep
    with whole-iteration handoff: T = MMs(1494) + ~50ns sem handoff +
    ACT(1106+285) + ~50ns, halved by double buffering => ~1650ns/iter.
  - All input DMAs go on the sync HWDGE ring; the scalar ring starves the
    matmul head.
  - reduce(i) must follow ts(i) immediately on the DVE queue; interleaving
    it behind ts(i+1) puts the reduce into the psum-rotation chain.
  - The compiler's end-of-program LoadActFuncSet restore (~2.7us post-
    barrier) is stripped: NRT preloads the exp table at NEFF load.

The host sums the [128, 56] per-core partials and does the exact O(batch)
margin/log math in float64.
"""

import math

import numpy as np

# Problem constants (hardcoded per contract; kernel.py must be self-contained)
NUM_CLASSES = 100000
EMB_SIZE = 512  # d
BATCH = 512  # n
N_CORES = 8
MRG_ANGLE = 0.5
MRG_SCALE = 64.0
GRAD_SCALE = 1.0

M0 = 64.0  # fixed logsumexp shift; |logit| <= s = 64 always
C_PAD = 100352  # = 8 * 12544
C_LOCAL = C_PAD // N_CORES  # 12544
N_PAD_ROWS = C_PAD - NUM_CLASSES  # 352 zero rows, all in core 7's shard

# 7 uniform slots per core (1792 classes each); each (slot, batch-row-tile)
# pair is one pipeline iteration over one of the two 4-bank psum halves.
# NOTE: PE writes and ACT/DVE reads must never touch the same psum half
# concurrently (mid-iteration reads or >2-deep rotations hang real HW), so
# the pipeline is exactly 2-deep with whole-iteration handoff.
SLOT_BOUNDS = [1792 * k for k in range(7)] + [12544]
N_SLOTS = len(SLOT_BOUNDS) - 1
N_TILES = BATCH // 128  # 4
N_ITERS = N_SLOTS * N_TILES  # 28
DEPTH = 2  # psum halves
ACT_FRAC_NUM, ACT_FRAC_DEN = 9, 14  # ScalarE handles 1152/1792 of each slot

# weight slabs (one dram tensor + one DMA each); all multiples of 512 so
# matmul chunks never cross a slab or a psum bank
SLABS = [512, 512, 768, 1792, 1792, 1792, 1792, 1792, 1792]
assert sum(SLABS) == C_LOCAL
_SLAB_OFF = [sum(SLABS[:i]) for i in range(len(SLABS))]

N_WARMUP = 10  # PE p-state ramp matmuls (more would outlast the ent DMA)

# HW-bisect knobs: per-chunk sem_pe (mid-iteration ACT/ts start) and
# chunk-inner P ordering both passed CoreSim but hung real HW; both off
# reverts to the proven per-iteration sync with the reduce off-chain.
PE_CHUNK_SEMS = False
P_INNER = False

# Schraudolph exp constants: exp(z - 64) ~= bitcast_f32(int32(z*SCH_A + SCH_C))
_LOG2E = 1.4426950408889634
_SIGMA = 0.045  # mantissa-approx bias tuning
SCH_A = float((1 << 23) * _LOG2E)
SCH_C = float((1 << 23) * (127.0 - _SIGMA - M0 * _LOG2E))

_COS_M = math.cos(MRG_ANGLE)
_SIN_M = math.sin(MRG_ANGLE)
_MM = math.sin(math.pi - MRG_ANGLE) * MRG_ANGLE
_THRESHOLD = math.cos(math.pi - MRG_ANGLE)
_PAD_FIX = N_PAD_ROWS * math.exp(-M0)  # pad rows contribute ~exp(0 - 64) each

_CACHED_NC = {}


def _slab_of(col):
    for si in range(len(SLABS) - 1, -1, -1):
        if col >= _SLAB_OFF[si]:
            return si, col - _SLAB_OFF[si]
    raise ValueError(col)


def _chunks_of_slot(g0, g1):
    """Split [g0,g1) at 512-col grid (psum-bank) and slab boundaries.

    Returns (c0, c1, slab_idx, off_in_slab) with c relative to g0."""
    cuts = {g0, g1}
    c = g0 + 512  # psum-bank grid is local to the slot's psum tile
    while c < g1:
        cuts.add(c)
        c += 512
    for off in _SLAB_OFF:
        if g0 < off < g1:
            cuts.add(off)
    cuts = sorted(cuts)
    out = []
    for a, b in zip(cuts[:-1], cuts[1:]):
        si, off = _slab_of(a)
        assert b - a <= 512 and off + (b - a) <= SLABS[si]
        out.append((a - g0, b - g0, si, off))
    return out


def build_nc():
    """Build the SPMD raw-bass program (one NEFF, run on all 8 cores)."""
    import concourse.bacc as bacc
    import concourse.mybir as mybir

    f32 = mybir.dt.float32
    bf16 = mybir.dt.bfloat16
    i32 = mybir.dt.int32
    f8 = mybir.dt.float8e4
    AF = mybir.ActivationFunctionType
    OP = mybir.AluOpType
    DR = mybir.MatmulPerfMode.DoubleRow
    AXX = mybir.AxisListType.X

    nc = bacc.Bacc(
        "TRN2", target_bir_lowering=False, debug=False, num_devices=N_CORES
    )

    # dram tensors
    ktn_dram = [
        nc.dram_tensor(f"kt{si}", [128, 4, W], f8, kind="ExternalInput")
        for si, W in enumerate(SLABS)
    ]
    # ent[p, P, i, n] = 8*e_n.T[P*256 + i*128 + p, n]
    ent_dram = nc.dram_tensor("ent", [128, 2, 2, BATCH], f8, kind="ExternalInput")
    sloc_out = nc.dram_tensor("sloc", [128, 2 * N_ITERS], f32, kind="ExternalOutput")

    # sbuf / psum
    ent_sb = nc.alloc_sbuf_tensor("ent_sb", [128, 2, 2, BATCH], f8)
    kt_sb = [
        nc.alloc_sbuf_tensor(f"kt_sb{si}", [128, 4, W], f8)
        for si, W in enumerate(SLABS)
    ]
    warm = nc.alloc_sbuf_tensor("warm", [128, 2, 128], f8)
    cneg64 = nc.alloc_sbuf_tensor("cneg64", [128, 1], f32)
    sacc = nc.alloc_sbuf_tensor("sacc", [128, 2 * N_ITERS], f32)
    wmax = max(g1 - g0 for g0, g1 in zip(SLOT_BOUNDS[:-1], SLOT_BOUNDS[1:]))
    wa_max = wmax * ACT_FRAC_NUM // ACT_FRAC_DEN
    ex = [
        nc.alloc_sbuf_tensor(f"ex{b}", [128, wa_max], bf16) for b in range(DEPTH)
    ]
    ui = [
        nc.alloc_sbuf_tensor(f"ui{b}", [128, wmax - wa_max], i32)
        for b in range(DEPTH)
    ]
    ps = [
        nc.alloc_psum_tensor(f"ps{b}", [128, 2048], f32) for b in range(DEPTH)
    ]

    def vps(i, c0, c1):
        # one 4-bank psum half per in-flight iteration; never mix PE
        # writes with ACT/DVE reads inside one half (hangs real HW)
        return ps[i % DEPTH][:, c0:c1]

    # semaphores
    sem_ent = nc.alloc_semaphore("sem_ent")
    sem_slab = [nc.alloc_semaphore(f"sem_slab{si}") for si in range(len(SLABS))]
    sem_init = nc.alloc_semaphore("sem_init")
    sem_pe = nc.alloc_semaphore("sem_pe")
    sem_acf = nc.alloc_semaphore("sem_acf")
    sem_vef = nc.alloc_semaphore("sem_vef")
    sem_red = nc.alloc_semaphore("sem_red")
    sem_out = nc.alloc_semaphore("sem_out")

    iters = [
        (slot, t) for slot in range(N_SLOTS) for t in range(N_TILES)
    ]  # iter index i = slot * N_TILES + t

    def slot_widths(slot):
        g0, g1 = SLOT_BOUNDS[slot], SLOT_BOUNDS[slot + 1]
        w = g1 - g0
        wa = w * ACT_FRAC_NUM // ACT_FRAC_DEN
        return g0, g1, w, wa

    # sem_pe counts completed (P1-stopped) psum chunks; pe_base[i] = count
    # before iter i, pe_act[i] = count at which ACT(i)'s columns are ready
    pe_base = [0]
    pe_act = []
    for slot, t in iters:
        g0, g1, w, wa = slot_widths(slot)
        chunks = _chunks_of_slot(g0, g1)
        cum = 0
        k = len(chunks)
        for ci, (c0, c1, si, off) in enumerate(chunks):
            if c1 >= wa and cum == 0:
                cum = 1
                k = ci + 1
        pe_act.append(pe_base[-1] + k)
        pe_base.append(pe_base[-1] + len(chunks))

    with nc.Block(no_gpsimd_drain=True) as block:

        @block.vector
        def _(vector):
            vector.memset(warm[:, :, :], 0.0).then_inc(sem_init)
            vector.memset(cneg64[:, :], -M0).then_inc(sem_init)

            def emit_ts(i):
                slot = iters[i][0]
                g0, g1, w, wa = slot_widths(slot)
                ts = vector.tensor_scalar(
                    ui[i % DEPTH][:, : w - wa],
                    vps(i, wa, w),
                    SCH_A,
                    SCH_C,
                    OP.mult,
                    OP.add,
                )
                ts._wait_ge(sem_pe, pe_base[i + 1] if PE_CHUNK_SEMS else i + 1)
                ts.then_inc(sem_vef)  # psum buffer release (DVE side)

            def emit_red(i):
                slot = iters[i][0]
                g0, g1, w, wa = slot_widths(slot)
                red = vector.reduce_sum(
                    sacc[:, N_ITERS + i : N_ITERS + i + 1],
                    ui[i % DEPTH][:, : w - wa].bitcast(f32),
                    axis=AXX,
                )
                red._wait_ge(sem_vef, i + 1)
                red.then_inc(sem_red)

            # natural order: red(i) right after ts(i) (the ~50ns same-queue
            # completion handoff is cheap); interleaving ts(i+1) first would
            # head-block red(i) behind the next iteration's matmuls and put
            # the reduce back into the psum-rotation critical chain
            for i in range(N_ITERS):
                emit_ts(i)
                emit_red(i)

        @block.scalar
        def _(scalar):
            for i, (slot, t) in enumerate(iters):
                g0, g1, w, wa = slot_widths(slot)
                a = scalar.activation(
                    ex[i % DEPTH][:, :wa],
                    vps(i, 0, wa),
                    AF.Exp,
                    bias=cneg64[:, :],
                    scale=1.0,
                    accum_out=sacc[:, i : i + 1],
                )
                a._wait_ge(sem_pe, pe_act[i] if PE_CHUNK_SEMS else i + 1)
                a.then_inc(sem_acf)

        @block.sync
        def _(sync):
            # all input DMAs on the sync HWDGE ring: the scalar ring measures
            # far slower and starves the matmul head
            sync.dma_start(out=ent_sb[:, :, :, :], in_=ent_dram[:, :, :, :]).then_inc(
                sem_ent, 16
            )
            for si in range(len(SLABS)):
                sync.dma_start(
                    out=kt_sb[si][:, :, :], in_=ktn_dram[si][:, :, :]
                ).then_inc(sem_slab[si], 16)
            sync.wait_ge(sem_acf, N_ITERS)
            od = sync.dma_start(
                out=sloc_out[:, :], in_=sacc[:, :], single_packet=True
            )
            od._wait_ge(sem_red, N_ITERS)
            od.then_inc(sem_out, 16)
            sync.wait_ge(sem_out, 16)

        @block.tensor
        def _(tensor):
            for wi in range(N_WARMUP):
                mm = tensor.matmul(
                    ps[0][:, :128],
                    lhsT=warm[:, :, :],
                    rhs=warm[:, :, :],
                    start=True,
                    stop=True,
                    perf_mode=DR,
                )
                if wi == 0:
                    mm._wait_ge(sem_init, 2)
            tensor.wait_ge(sem_ent, 16)
            slabs_seen = set()
            for i, (slot, t) in enumerate(iters):
                g0, g1, w, wa = slot_widths(slot)
                chunks = _chunks_of_slot(g0, g1)
                n_ch = len(chunks)
                if P_INNER:
                    order = [(ci, P) for ci in range(n_ch) for P in (0, 1)]
                else:
                    order = [(ci, P) for P in (0, 1) for ci in range(n_ch)]
                emitted_first = False
                for ci, P in order:
                    c0, c1, si, off = chunks[ci]
                    new_slab = P == 0 and si not in slabs_seen
                    if new_slab:
                        slabs_seen.add(si)
                    if not emitted_first and i >= DEPTH:
                        # one wait slot per instruction: ACT-chain wait goes
                        # standalone, DVE-chain (or slab) inline
                        tensor.wait_ge(sem_acf, i - DEPTH + 1)
                        if new_slab:
                            tensor.wait_ge(sem_vef, i - DEPTH + 1)
                    mm = tensor.matmul(
                        vps(i, c0, c1),
                        lhsT=ent_sb[:, P, :, t * 128 : (t + 1) * 128],
                        rhs=kt_sb[si][:, 2 * P : 2 * P + 2, off : off + (c1 - c0)],
                        start=(P == 0),
                        stop=(P == 1),
                        perf_mode=DR,
                    )
                    if new_slab:
                        mm._wait_ge(sem_slab[si], 16)
                    elif not emitted_first and i >= DEPTH:
                        mm._wait_ge(sem_vef, i - DEPTH + 1)
                    emitted_first = True
                    is_last = (ci, P) == order[-1]
                    closes_chunk = P == 1
                    if is_last:
                        if i >= DEPTH:
                            # ui WAR chain: ts(i+DEPTH) inherits red(i) via
                            # sem_pe set by this instruction
                            mm._wait_ge(sem_red, i - DEPTH + 1)
                        mm.then_inc(sem_pe)
                    elif PE_CHUNK_SEMS and closes_chunk:
                        mm.then_inc(sem_pe)

    nc.compile()
    # Drop the end-of-program ACT-table restore: the entry-side table load is
    # NRT's pseudo-load before the first ACTIVATE; the trailing restore only
    # adds ~2.7us of post-barrier tail to every execution.
    for blk in nc.main_func.blocks:
        blk.instructions[:] = [
            ins
            for ins in blk.instructions
            if not isinstance(ins, mybir.InstLoadActFuncSet)
        ]
    return nc


def get_nc():
    if "nc" not in _CACHED_NC:
        _CACHED_NC["nc"] = build_nc()
    return _CACHED_NC["nc"]


def make_in_maps(embeddings, kernel, label):
    """Host-side sharding / layout prep -> per-core input maps."""
    import ml_dtypes

    f8 = ml_dtypes.float8_e4m3

    e = np.asarray(embeddings, dtype=np.float32)
    k = np.asarray(kernel, dtype=np.float32)

    kn = (k / np.linalg.norm(k, axis=1, keepdims=True)).astype(np.float32)
    en = (e / np.linalg.norm(e, axis=1, keepdims=True)).astype(np.float32)
    # s = 64 split as 8 * 8 across the two fp8 matmul operands
    ent8 = np.ascontiguousarray(
        (8.0 * en).T.astype(f8).reshape(2, 2, 128, BATCH).transpose(2, 0, 1, 3)
    )

    knp = np.zeros((C_PAD, EMB_SIZE), f8)
    knp[:NUM_CLASSES] = (8.0 * kn).astype(f8)
    # [d, C] -> [128, 4, C]: row p holds the four d-chunk slices (d=j*128+p)
    knT = knp.T.reshape(4, 128, C_PAD).transpose(1, 0, 2)

    in_maps = []
    for r in range(N_CORES):
        sh = knT[:, :, r * C_LOCAL : (r + 1) * C_LOCAL]
        m = {"ent": ent8}
        c0 = 0
        for si, W in enumerate(SLABS):
            m[f"kt{si}"] = np.ascontiguousarray(sh[:, :, c0 : c0 + W])
            c0 += W
        in_maps.append(m)
    return in_maps, en, kn


def finish_host(results, en, kn, label):
    """Combine per-core partials into the scalar loss (gather/unshard).

    The margin path is exact O(batch) math: 512 dot products of length 512
    plus elementwise trig, all in float64."""
    lab = np.asarray(label).reshape(-1).astype(np.int64)
    S = np.zeros((128, N_TILES), np.float64)
    for r in range(N_CORES):
        sl = results[r]["sloc"].astype(np.float64)  # [128, 64]
        both = sl[:, :N_ITERS] + sl[:, N_ITERS:]
        S += both.reshape(128, N_SLOTS, N_TILES).sum(axis=1)
    S = S.T.reshape(-1)  # [n] in row order: n = t*128 + p -> transpose

    zy = MRG_SCALE * np.einsum(
        "nd,nd->n", en.astype(np.float64), kn[lab].astype(np.float64)
    )
    cos_t = zy / MRG_SCALE
    sin_t = np.sqrt(np.maximum(1.0 - cos_t * cos_t, 0.0))
    new_zy = MRG_SCALE * (cos_t * _COS_M - sin_t * _SIN_M)
    zy_keep = zy - MRG_SCALE * _MM
    zyf = np.where(cos_t - _THRESHOLD > 0, new_zy, zy_keep)

    adj = S - _PAD_FIX + np.exp(zyf - M0) - np.exp(zy - M0)
    nll = np.log(adj) + M0 - zyf
    return np.float32(GRAD_SCALE * nll.mean())


def kernel(embeddings, kernel, label):
    from concourse.bass_utils import run_bass_kernel_spmd

    in_maps, en, kn = make_in_maps(embeddings, kernel, label)
    nc = get_nc()
    res = run_bass_kernel_spmd(nc, in_maps, core_ids=list(range(N_CORES)))
    return finish_host(res.results, en, kn, label)
